# revision 52
# baseline (speedup 1.0000x reference)
"""Multi-head self-attention (B=4, S=2048, D=1024, H=16) on 8 NeuronCores.

Sharding: data-parallel over batch (4 groups) x tensor-parallel over heads
(2 groups of 8 heads).  Core c handles batch b=c//2, head-group g=c%2.
Each core computes its 8 heads' attention plus a partial out-projection;
the host sums the two partials per batch, transposes, adds out_b.

v2 design (cost-model driven):
  - projection inputs (x, wq, wk, wv, wo) in bf16: same PE rate as fp32r
    in the cost model, half the HBM traffic; attention core (qkT, v, pt)
    stays fp32r for precision
  - all intermediates SBUF-resident (v, otn): no DRAM bounces between
    phases; out-projection folded into the main loop
  - single-head attention slots: PSUM = scores dbuf (4 banks) + AV
    accumulator (2) + two 1-bank rings for interleaved projections
  - software-pipelined emission: scores(i+1) and AV(i) interleave so the
    PE never waits on the ACT-bound exp chain; deferred projection
    matmuls (qk pairs 1-3, v pairs 2-3, out-proj chunk 0) fill the
    remaining ACT-paced gaps via a deadline-driven scheduler
  - PE pre-warm dummy matmuls during the DMA lead-in (p-state ramp)
  - denominators ride the AV matmul as row 64 (ones column of the
    augmented v); normalization via DRAM-bounce partition broadcast
"""

import numpy as np

_B, _S, _D, _H = 4, 2048, 1024, 16
_FH = 512                # local feature dims per core (8 heads x 64)
_ND = _D // 128          # contraction tiles for projections
_NPAIR = _FH // 128      # head pairs per core
_NH = _FH // 64          # local heads
_FHA = _NH * 65          # v width incl. per-head ones column
_NCORES = 8

_CACHE = {}


def _build(S):
    import concourse.bass as bass
    import concourse.bacc as bacc
    import concourse.tile as tile
    import concourse.mybir as mybir
    from contextlib import ExitStack
    import itertools

    f32 = mybir.dt.float32
    f32r = mybir.dt.float32r
    bf16 = mybir.dt.float16
    Exp = mybir.ActivationFunctionType.Exp
    D, FH, ND, NPAIR, NH, FHA = _D, _FH, _ND, _NPAIR, _NH, _FHA
    NKT = S // 128           # key tiles
    CH = 1024                # query chunk
    NCH = S // CH            # 2
    HW = 512                 # matmul moving free dim
    NHALF = CH // HW         # 2
    TS = 512                 # projection token slice
    NTS = S // TS            # 4

    nc = bacc.Bacc("TRN2", target_bir_lowering=False, debug=False)

    xT_d = nc.dram_tensor("xT", [D, S], bf16, kind="ExternalInput")
    wq_d = nc.dram_tensor("wq", [NPAIR, 128, ND, 128], bf16, kind="ExternalInput")
    wk_d = nc.dram_tensor("wk", [NPAIR, 128, ND, 128], bf16, kind="ExternalInput")
    wv_d = nc.dram_tensor("wv", [128, ND, FHA], bf16, kind="ExternalInput")
    wo_d = nc.dram_tensor("wo", [128, NPAIR, D], bf16, kind="ExternalInput")
    bq_d = nc.dram_tensor("bq", [128, NPAIR], f32, kind="ExternalInput")
    bk_d = nc.dram_tensor("bk", [128, NPAIR], f32, kind="ExternalInput")
    bv_d = nc.dram_tensor("bv", [1, FHA], bf16, kind="ExternalInput")
    outp_d = nc.dram_tensor("outp", [ND, NTS, 128, TS], bf16,
                            kind="ExternalOutput")

    with tile.TileContext(nc) as tc, ExitStack() as top:
        consts = top.enter_context(tc.tile_pool(name="consts", bufs=1))
        xtp = top.enter_context(tc.tile_pool(name="xtp", bufs=1))
        wqkp = top.enter_context(tc.tile_pool(name="wqkp", bufs=1))
        wvp = top.enter_context(tc.tile_pool(name="wvp", bufs=1))
        qkp = top.enter_context(tc.tile_pool(name="qkp", bufs=1))
        vp = top.enter_context(tc.tile_pool(name="vp", bufs=1))
        otnp = top.enter_context(tc.tile_pool(name="otnp", bufs=1))
        wop = top.enter_context(tc.tile_pool(name="wop", bufs=1))
        ps = top.enter_context(tc.tile_pool(name="ps", bufs=1, space="PSUM"))
        ptp = top.enter_context(tc.tile_pool(name="ptp", bufs=2))
        nrm = top.enter_context(tc.tile_pool(name="nrm", bufs=1))
        stp = top.enter_context(tc.tile_pool(name="stp", bufs=2))
        drp = top.enter_context(tc.tile_pool(name="drp", bufs=2, space="DRAM"))

        # ---- DMA issue: priority order, alternating HWDGE queues ----
        engs = itertools.cycle([nc.sync, nc.scalar])

        def dma(out, in_):
            next(engs).dma_start(out=out, in_=in_)

        # memset consts need no DMA: warmup can start immediately
        ones_row = consts.tile([1, 128], bf16)
        nc.vector.memset(ones_row, 1.0)
        ones_r = consts.tile([128, 64], f32)
        nc.vector.memset(ones_r, 1.0)
        # DMA transfers serialize globally (HBM bandwidth) and each
        # dma_start costs its issuing SEQ ~600ns, so the two transfers the
        # first v-projection needs (wv phase-A columns, first x tile) lead
        # their queues; everything else queues behind by first-use order
        xT_dv = xT_d.reshape([ND, 128, S])
        wv_sb = wvp.tile([128, ND, FHA], bf16)
        nc.sync.dma_start(out=wv_sb[:, :, 0:260], in_=wv_d[:, :, 0:260])
        xT_sb = xtp.tile([128, ND, S], bf16)
        XC = 512

        def xcols(lo, hi, eng):
            eng.dma_start(
                out=xT_sb[:, :, lo:hi],
                in_=xT_dv[:, :, lo:hi].transpose([1, 0, 2]))

        xcols(0, 128, nc.scalar)
        bv_sb = consts.tile([1, FHA], bf16)
        nc.sync.dma_start(out=bv_sb, in_=bv_d[:])
        xcols(128, 512, nc.sync)
        bqk_sb = consts.tile([128, 2 * NPAIR], f32)
        nc.scalar.dma_start(out=bqk_sb[:, 0:NPAIR], in_=bq_d[:])
        nc.scalar.dma_start(out=bqk_sb[:, NPAIR:2 * NPAIR], in_=bk_d[:])
        # exp table load during the ramp, not at the first real softmax exp
        warm = consts.tile([1, 8], f32)
        nc.vector.memset(warm, 0.0)
        nc.scalar.activation(out=warm, in_=warm, func=Exp)
        wqk_sb = wqkp.tile([128, NPAIR, 2, ND, 128], bf16)
        nc.scalar.dma_start(out=wqk_sb[:, 0, 0], in_=wq_d[0])
        nc.scalar.dma_start(out=wqk_sb[:, 0, 1], in_=wk_d[0])
        xcols(512, 1024, nc.sync)
        nc.sync.dma_start(out=wv_sb[:, :, 260:520], in_=wv_d[:, :, 260:520])
        xcols(1024, 1536, nc.scalar)
        xcols(1536, 2048, nc.sync)
        for p in range(1, NPAIR):
            dma(wqk_sb[:, p, 0], wq_d[p])
            dma(wqk_sb[:, p, 1], wk_d[p])
        wo_sb = wop.tile([128, NPAIR, D], bf16)
        dma(wo_sb, wo_d[:])

        qkT = qkp.tile([128, NPAIR, 2, S], f32r)       # [f%128, pair, q/k, t]
        v_sb = vp.tile([128, NKT, FHA], f32r)          # [key%128, ktile, feat]
        otn_sb = otnp.tile([128, NPAIR, S], bf16)      # [feat%128, pair, t]

        # ---- PE pre-warm: ramp the p-state during the DMA lead-in ----
        dmy = ps.tile([128, 128], f32, tag="pq", bufs=1)
        for _ in range(14):
            nc.tensor.matmul(dmy, lhsT=ones_row, rhs=ones_row,
                             start=True, stop=True)

        # ---- emission helpers ----
        def vproj(t, c0, cw, vtag):
            """v columns [c0, c0+cw) for token tile t -> v_sb."""
            vps = ps.tile([128, cw], f32, tag=vtag, bufs=2)
            for d in range(ND):
                nc.tensor.matmul(
                    vps,
                    lhsT=xT_sb[:, d, t * 128:(t + 1) * 128],
                    rhs=wv_sb[:, d, c0:c0 + cw],
                    start=(d == 0), stop=False,
                )
            nc.tensor.matmul(vps, lhsT=ones_row, rhs=bv_sb[:, c0:c0 + cw],
                             start=False, stop=True)
            nc.vector.tensor_copy(out=v_sb[:, t, c0:c0 + cw], in_=vps)

        def qkproj(p, which, j):
            pps = ps.tile([128, TS], f32, tag="pq", bufs=1)
            for d in range(ND):
                nc.tensor.matmul(
                    pps,
                    lhsT=wqk_sb[:, p, which, d, :],
                    rhs=xT_sb[:, d, j * TS:(j + 1) * TS],
                    start=(d == 0), stop=(d == ND - 1),
                )
            nc.vector.tensor_scalar_add(
                out=qkT[:, p, which, j * TS:(j + 1) * TS],
                in0=pps,
                scalar1=bqk_sb[:, which * NPAIR + p:which * NPAIR + p + 1],
            )

        # ---- phase A: v pairs 0-1 + qk pair 0, paced behind the x DMA ----
        for c in range(S // XC):
            for t in range(4 * c, 4 * c + 4):
                vproj(t, 0, 260, "s")
            qkproj(0, 1, c)
            if c < 2:
                qkproj(0, 0, c)

        # ---- deferred-work scheduler ----
        # Each task is a list of closures (one PE matmul each, roughly);
        # (units, due_slot) with force-drain at slot starts, rate-based
        # fill between attention matmuls.
        pending = []     # list of [closure, due_slot]

        def push(units, due):
            for u in units:
                pending.append([u, due])

        def qk_units(p, which, jlist):
            units = []
            state = {}
            for j in jlist:
                def mk(d, j=j, which=which, p=p):
                    def go():
                        if d == 0:
                            state['pps'] = ps.tile(
                                [128, TS], f32, tag="pq", bufs=1, name="pps")
                        nc.tensor.matmul(
                            state['pps'],
                            lhsT=wqk_sb[:, p, which, d, :],
                            rhs=xT_sb[:, d, j * TS:(j + 1) * TS],
                            start=(d == 0), stop=(d == ND - 1),
                        )
                        if d == ND - 1:
                            nc.vector.tensor_scalar_add(
                                out=qkT[:, p, which, j * TS:(j + 1) * TS],
                                in0=state['pps'],
                                scalar1=bqk_sb[:, which * NPAIR + p:
                                               which * NPAIR + p + 1],
                            )
                    return go
                for d in range(ND):
                    units.append(mk(d))
            return units

        def vdef_units(t):
            state = {}
            units = []
            def mk(d, t=t):
                def go():
                    if d == 0:
                        state['vps'] = ps.tile(
                            [128, 260], f32, tag="op", bufs=1, name="vps")
                    if d < ND:
                        nc.tensor.matmul(
                            state['vps'],
                            lhsT=xT_sb[:, d, t * 128:(t + 1) * 128],
                            rhs=wv_sb[:, d, 260:520],
                            start=(d == 0), stop=False,
                        )
                    else:
                        nc.tensor.matmul(
                            state['vps'], lhsT=ones_row,
                            rhs=bv_sb[:, 260:520], start=False, stop=True)
                        nc.vector.tensor_copy(
                            out=v_sb[:, t, 260:520], in_=state['vps'])
                return go
            for d in range(ND + 1):
                units.append(mk(d))
            return units

        def outproj_units(ch, copy_engines=None):
            """out-proj tiles for query chunk ch: et x j, accumulate pairs."""
            units = []
            stpair = {}
            coff = ch * CH
            jlist = range(coff // TS, (coff + CH) // TS)
            ceng = itertools.cycle(copy_engines or [nc.vector])
            oeng = itertools.cycle([nc.sync, nc.scalar]
                                   if copy_engines else [nc.sync])
            for et in range(ND):
                for j in jlist:
                    state = {}
                    def mk(p, et=et, j=j):
                        def go():
                            if p == 0:
                                state['ops'] = ps.tile(
                                    [128, TS], f32, tag="op", bufs=1,
                                    name="ops")
                            nc.tensor.matmul(
                                state['ops'],
                                lhsT=wo_sb[:, p, et * 128:(et + 1) * 128],
                                rhs=otn_sb[:, p, j * TS:(j + 1) * TS],
                                start=(p == 0), stop=(p == NPAIR - 1),
                            )
                            if p == NPAIR - 1:
                                if j % 2 == 0:
                                    stpair['t'] = stp.tile(
                                        [128, 2, TS], bf16, tag="st",
                                        bufs=4, name="st")
                                st = stpair['t']
                                eng = next(ceng)
                                if eng is nc.vector:
                                    eng.tensor_copy(out=st[:, j % 2, :],
                                                    in_=state['ops'])
                                else:
                                    eng.copy(out=st[:, j % 2, :],
                                             in_=state['ops'])
                                if j % 2 == 1:
                                    next(oeng).dma_start(
                                        out=outp_d[et, j - 1:j + 1]
                                        .transpose([1, 0, 2]),
                                        in_=st)
                        return go
                    for p in range(NPAIR):
                        units.append(mk(p))
            return units

        SLOTS = [(h, ch) for ch in range(NCH) for h in range(NH)]
        NSLOT = len(SLOTS)

        push(qk_units(1, 1, range(NTS)), 2)       # k pair1: due slot 2
        push(qk_units(1, 0, [0, 1]), 2)           # q pair1 chunk0
        push(qk_units(2, 1, range(NTS)), 4)
        push(qk_units(2, 0, [0, 1]), 4)
        for t in range(NKT):
            push(vdef_units(t), 4)                # v pairs 2-3: due slot 4
        push(qk_units(3, 1, range(NTS)), 6)
        push(qk_units(3, 0, [0, 1]), 6)
        push(qk_units(0, 0, [2, 3]), 8)           # q for chunk1 queries
        push(qk_units(1, 0, [2, 3]), 10)
        push(qk_units(2, 0, [2, 3]), 12)
        push(qk_units(3, 0, [2, 3]), 14)

        def drain_due(si):
            while pending and pending[0][1] <= si:
                pending.pop(0)[0]()

        def fill(si, pts_left):
            """Emit deferred units at a rate meeting every deadline."""
            if not pending:
                return
            # units required per remaining fill point, by deadline prefix
            need = 0
            cnt = 0
            best = 0.0
            for u, due in pending:
                cnt += 1
                pts = pts_left + max(0, due - si - 1) * NKT
                if pts <= 0:
                    need = max(need, cnt)
                    continue
                best = max(best, cnt / pts)
            n = max(need, int(best + 0.999))
            n = min(n, 6, len(pending))
            for _ in range(n):
                pending.pop(0)[0]()

        # ---- phase B: 16 single-head attention slots ----
        def slot(si, h, ch, last=False):
            p, hh = h // 2, h % 2
            coff = ch * CH
            r0, r1 = hh * 64, (hh + 1) * 64
            o = ps.tile([128, CH], f32, tag="o", bufs=1, name="o")
            pt_prev = None
            for i in range(NKT):
                s = ps.tile([128, CH], f32, tag="s", bufs=2, name="s")
                for half in range(NHALF):
                    q0 = coff + half * HW
                    nc.tensor.matmul(
                        s[:, half * HW:(half + 1) * HW],
                        lhsT=qkT[r0:r1, p, 1, i * 128:(i + 1) * 128],
                        rhs=qkT[r0:r1, p, 0, q0:q0 + HW],
                        start=True, stop=True,
                        tile_position=(r0, 0),
                    )
                pt = ptp.tile([128, CH], f32r, tag="pt", bufs=3, name="pt")
                nc.scalar.activation(out=pt, in_=s, func=Exp, scale=0.125)
                if i > 0:
                    va = v_sb[:, i - 1, p * 130 + hh * 65:
                              p * 130 + hh * 65 + 65]
                    for half in range(NHALF):
                        hs = slice(half * HW, (half + 1) * HW)
                        nc.tensor.matmul(
                            o[0:65, hs], lhsT=va, rhs=pt_prev[:, hs],
                            start=(i - 1 == 0), stop=False,
                        )
                pt_prev = pt
                fill(si, NKT - 1 - i)
            va = v_sb[:, NKT - 1, p * 130 + hh * 65:p * 130 + hh * 65 + 65]
            for half in range(NHALF):
                hs = slice(half * HW, (half + 1) * HW)
                nc.tensor.matmul(
                    o[0:65, hs], lhsT=va, rhs=pt_prev[:, hs],
                    start=False, stop=True,
                )
            # normalization: denominator is row 64 (ones column of v)
            aS = nrm.tile([65, CH], f32, tag="a", bufs=1, name="aS")
            if not last:
                nc.vector.tensor_copy(out=aS, in_=o[0:65, :])
            if last:
                # per-half pipelined chain: copy -> K=1 PE-matmul
                # broadcast of the raw denominator (on the pq/op rings,
                # free at the tail, so each half gates independently) ->
                # reciprocal -> multiply.  Broadcast-then-recip, not
                # recip-then-broadcast: a custom-DVE write is not
                # tracked as a PE-read dependency.
                rS = nrm.tile([64, CH], f32, tag="r", bufs=1, name="rS")
                rtags = ["pq", "op"]
                rp = []
                for half in range(NHALF):
                    hs = slice(half * HW, (half + 1) * HW)
                    nc.vector.tensor_copy(out=aS[:, hs], in_=o[0:65, hs])
                    rph = ps.tile([64, HW], f32, tag=rtags[half % 2],
                                  bufs=1, name="rph")
                    nc.tensor.matmul(
                        rph, lhsT=ones_r[64:65, :],
                        rhs=aS[64:65, hs], start=True, stop=True,
                        tile_position=(64, 0),
                    )
                    rp.append(rph)
                for half in range(NHALF):
                    hs = slice(half * HW, (half + 1) * HW)
                    nc.vector.reciprocal_approx_fast(
                        out=rS[:, hs], in_=rp[half])
                    nc.vector.tensor_mul(
                        out=otn_sb[r0:r1, p,
                                   coff + half * HW:coff + (half + 1) * HW],
                        in0=aS[0:64, hs], in1=rS[:, hs])
            else:
                dscr = drp.tile([1, CH], f32, tag="d", name="dscr")
                nc.sync.dma_start(out=dscr, in_=aS[64:65, :])
                rS = nrm.tile([64, CH], f32, tag="r", bufs=1, name="rS")
                nc.sync.dma_start(
                    out=rS, in_=dscr[0:1, :].to_broadcast([64, CH]))
                nc.vector.reciprocal_approx_fast(out=rS, in_=rS)
                nc.vector.tensor_mul(
                    out=otn_sb[r0:r1, p, coff:coff + CH],
                    in0=aS[0:64, :], in1=rS)

        for si, (h, ch) in enumerate(SLOTS):
            drain_due(si)
            if si == NH:                 # chunk 0 otn complete after slot 7
                push(outproj_units(0), NSLOT)
            slot(si, h, ch, last=(si == NSLOT - 1))
        drain_due(NSLOT)

        # ---- tail: out-projection for the last chunk ----
        # pairs 0-2 accumulate while the last slot's normalize drains;
        # rotating psum tags (attention tags are idle now) pipeline the
        # tiles: stage-1 of tile n+4 only after tile n's copy is emitted,
        # so the in-order PE stream never waits on a later-emitted copy
        coff = (NCH - 1) * CH
        tiles = [(et, j) for j in range(coff // TS, (coff + CH) // TS)
                 for et in range(ND)]
        ttags = [("s", 2), ("pq", 1), ("op", 1), ("o", 1)]
        ceng = itertools.cycle([nc.vector, nc.scalar])
        oeng = itertools.cycle([nc.sync, nc.scalar])
        opst = {}
        tl_stpair = {}

        def t_stage1(n):
            et, j = tiles[n]
            tg, tb = ttags[n % 4]
            opn = ps.tile([128, TS], f32, tag=tg, bufs=tb, name="opn")
            opst[n] = opn
            for p in range(NPAIR - 1):
                nc.tensor.matmul(
                    opn,
                    lhsT=wo_sb[:, p, et * 128:(et + 1) * 128],
                    rhs=otn_sb[:, p, j * TS:(j + 1) * TS],
                    start=(p == 0), stop=False,
                )

        def t_stage2(n):
            et, j = tiles[n]
            opn = opst.pop(n)
            p = NPAIR - 1
            nc.tensor.matmul(
                opn,
                lhsT=wo_sb[:, p, et * 128:(et + 1) * 128],
                rhs=otn_sb[:, p, j * TS:(j + 1) * TS],
                start=False, stop=True,
            )
            if n % 2 == 0:
                tl_stpair['t'] = stp.tile([128, 2, TS], bf16, tag="st",
                                          bufs=4, name="st")
            st = tl_stpair['t']
            eng = next(ceng)
            if eng is nc.vector:
                eng.tensor_copy(out=st[:, n % 2, :], in_=opn)
            else:
                eng.copy(out=st[:, n % 2, :], in_=opn)
            if n % 2 == 1:
                next(oeng).dma_start(
                    out=outp_d[et - 1:et + 1, j].transpose([1, 0, 2]),
                    in_=st)

        for n in range(4):
            t_stage1(n)
        for n in range(len(tiles)):
            t_stage2(n)
            if n + 4 < len(tiles):
                t_stage1(n + 4)

    nc.compile()
    return nc


def _get_nc(S=_S):
    if S not in _CACHE:
        _CACHE[S] = _build(S)
    return _CACHE[S]


def _bf16(a):
    return np.ascontiguousarray(np.asarray(a, dtype=np.float32)
                                .astype(np.float16))


def _c32(a):
    return np.ascontiguousarray(a, dtype=np.float32)


def make_in_map(xT, wqT, wkT, wvT, woT, bq, bk, bv):
    """Pack one core's inputs into the kernel's tiled DRAM layouts.

    xT: [D, S] (x transposed); wqT/wkT/wvT: [D, FH] (W sections
    transposed); woT: [FH, D] (out_w columns transposed); biases: [FH].
    """
    D, FH, ND, NPAIR, NH, FHA = _D, _FH, _ND, _NPAIR, _NH, _FHA
    # augment v with a per-head ones column: wv gets zero columns, bv gets
    # 1.0 entries -> the bias matmul produces the ones column, whose AV
    # accumulation yields the softmax denominators for free
    wva = np.zeros((D, FHA), dtype=np.float32)
    bva = np.zeros((1, FHA), dtype=np.float32)
    for h in range(NH):
        wva[:, h * 65:h * 65 + 64] = np.asarray(wvT)[:, h * 64:(h + 1) * 64]
        bva[0, h * 65:h * 65 + 64] = np.asarray(bv)[h * 64:(h + 1) * 64]
        bva[0, h * 65 + 64] = 1.0
    return {
        "xT": _bf16(xT),
        "wq": _bf16(np.asarray(wqT).reshape(ND, 128, NPAIR, 128)
                    .transpose(2, 1, 0, 3)),
        "wk": _bf16(np.asarray(wkT).reshape(ND, 128, NPAIR, 128)
                    .transpose(2, 1, 0, 3)),
        "wv": _bf16(wva.reshape(ND, 128, FHA).transpose(1, 0, 2)),
        "wo": _bf16(np.asarray(woT).reshape(NPAIR, 128, D).transpose(1, 0, 2)),
        "bq": _c32(np.asarray(bq).reshape(_NPAIR, 128).T),
        "bk": _c32(np.asarray(bk).reshape(_NPAIR, 128).T),
        "bv": _bf16(bva),
    }


def unpack_out(outp_tiled, S=_S):
    """[ND, NTS, 128, TS] tiled partial -> [D, S]."""
    return (outp_tiled.astype(np.float32)
            .transpose(0, 2, 1, 3).reshape(_D, S))


def _shard_inputs(x, in_proj_weight, in_proj_bias, out_w):
    w = np.asarray(in_proj_weight)
    b = np.asarray(in_proj_bias)
    ow = np.asarray(out_w)
    in_maps = []
    for c in range(_NCORES):
        bi, g = divmod(c, 2)
        sl = slice(g * _FH, (g + 1) * _FH)
        in_maps.append(make_in_map(
            xT=np.asarray(x[bi]).T,
            wqT=w[0 * _D:1 * _D][sl].T,
            wkT=w[1 * _D:2 * _D][sl].T,
            wvT=w[2 * _D:3 * _D][sl].T,
            woT=ow[:, sl].T,
            bq=b[0 * _D:1 * _D][sl],
            bk=b[1 * _D:2 * _D][sl],
            bv=b[2 * _D:3 * _D][sl],
        ))
    return in_maps


LAST_RESULTS = None


def kernel(x, in_proj_weight, in_proj_bias, out_w, out_b):
    global LAST_RESULTS
    from concourse.bass_utils import run_bass_kernel_spmd
    import os

    nc = _get_nc()
    in_maps = _shard_inputs(x, in_proj_weight, in_proj_bias, out_w)
    trace = os.environ.get("BASS_TRACE", "0") not in ("", "0")
    res = run_bass_kernel_spmd(
        nc, in_maps, core_ids=list(range(_NCORES)), trace=trace
    )
    LAST_RESULTS = res
    out_b = np.asarray(out_b, dtype=np.float32)
    out = np.empty((_B, _S, _D), dtype=np.float32)
    for b in range(_B):
        part = (unpack_out(res.results[2 * b]["outp"])
                + unpack_out(res.results[2 * b + 1]["outp"]))
        out[b] = part.T + out_b
    return out
